# revision 1
# baseline (speedup 1.0000x reference)
"""Trainium2 Bass kernel for nn_ByteFormerWrapper (block_size=4096).

Math: reference computes img = byte2image_4k(x) (B,8,128,496) then
out = einsum('bchw,wo->bcho', img, W).

Key identity: img[b, c, p*8+s, i] = shifts_s[b, c, i+p] where
shifts_s[b, c, j] = ((F >> (8-s)) & 255), F = 256*x[b,512c+j] + x[b,512c+j+1]
(next byte zero at j=511, per 512-byte sub-block), for i in [0,496),
p in [0,16), s in [0,8). Since i+p <= 510 there is no wraparound.

So with norm(v) = v*(2/255) - 1:
  out[b,c,p*8+s,o] = sum_j shifts_s[b,c,j] * Wsc_p[j,o] - S[o]
where Wsc_p is W*(2/255) zero-padded to 512 rows at offset p, S = W.sum(0).

Device program (per core, 32 batch rows => 256 (b,c) sub-blocks):
  per j-chunk k (128 j's): XT[k][j,bc], XT1[k][j,bc] (next byte) loaded
  via int16 DMA transpose from DRAM; F = (XT<<8)|XT1 on DVE.
  per (k, s): at[k][j, s*256+bc] = f16((F >> (8-s)) & 255)  (DVE, 2 ops)
  matmul: stationary = [Wsc_{2q} | Wsc_{2q+1}] chunk [128j, 128m] f16,
  moving = at[k][:, 512n:512n+512] (n-chunk = s pair); accumulate k=0..3
  into PSUM [128,512] f32; ACT eviction adds -S; DMA to OT [16,64,2048]
  = [p, o, (s,bc)].
Host reassembles OT -> (32,8,128,64) per core, concat over 8 cores.
"""

import numpy as np

NCORES = 8
B = 256
B_LOC = B // NCORES  # 32 batch rows per core
SUB = 512

_CACHE = {}


def _build_program(repeat=1):
    import concourse.mybir as mybir
    import concourse.tile as tile
    from concourse import bacc

    f32 = mybir.dt.float32
    f16 = mybir.dt.float16
    i16 = mybir.dt.int16
    Alu = mybir.AluOpType

    nc = bacc.Bacc(None, target_bir_lowering=False, debug=False)

    with tile.TileContext(nc) as tc:
        with tc.tile_pool(name="dram", bufs=1, space="DRAM") as dram:
            x_d = dram.tile([256, 512], i16, kind="ExternalInput", name="x", uniquify=False)
            ws_d = dram.tile([128, 4096], f16, kind="ExternalInput", name="ws", uniquify=False)
            bias_d = dram.tile([128, 1], f32, kind="ExternalInput", name="bias", uniquify=False)
            ot_d = dram.tile([16, 64, 2048], f32, kind="ExternalOutput", name="ot", uniquify=False)
            ot_flat = ot_d.rearrange("p o n -> (p o) n")
            ot4 = ot_flat.rearrange("(g qq pp) n -> g pp qq n", g=2, qq=4)

            with (
                tc.tile_pool(name="const", bufs=1) as constp,
                tc.tile_pool(name="xin", bufs=2) as xinp,
                tc.tile_pool(name="sh", bufs=8) as shp,
                tc.tile_pool(name="at", bufs=2) as atp,
                tc.tile_pool(name="mpsum", bufs=8, space="PSUM") as mpsum,
                tc.tile_pool(name="oev", bufs=6) as oevp,
            ):
                bias_sb = constp.tile([128, 1], f32)
                nc.scalar.dma_start(bias_sb[:], bias_d[:])
                ws_sbk = []
                for kk in range(4):
                    w1 = constp.tile([128, 1024], f16, name=f"ws_sb{kk}")
                    nc.scalar.dma_start(w1[:], ws_d[:, 1024 * kk:1024 * (kk + 1)])
                    ws_sbk.append(w1)

                def body():
                    at = [atp.tile([128, 2048], f16, name=f"at{k}") for k in range(4)]
                    ps0 = [mpsum.tile([128, 512], f32, name="ps", tag="ps") for _ in range(8)]
                    def extract(k, s):
                        sht_i = shp.tile([128, 256], i16, name="sht_i")
                        nc.vector.tensor_scalar(
                            sht_i[:], Fs[k][:], 8 - s, 255,
                            op0=Alu.logical_shift_right,
                            op1=Alu.bitwise_and,
                        )
                        nc.vector.tensor_copy(at[k][:, 256 * s:256 * (s + 1)], sht_i[:])

                    def mm(q, n, k, ps):
                        nc.tensor.matmul(
                            ps[:],
                            ws_sbk[k][:, q * 128:q * 128 + 128],
                            at[k][:, 512 * n:512 * (n + 1)],
                            start=(k == 0),
                            stop=(k == 3),
                        )

                    def evict(q, n, ps, ev, dve=False):
                        if dve:
                            nc.vector.tensor_scalar(
                                ev[:, q % 4, :], ps[:], bias_sb[:], None, op0=Alu.add)
                        else:
                            nc.scalar.activation(
                                ev[:, q % 4, :], ps[:],
                                mybir.ActivationFunctionType.Identity,
                                bias=bias_sb[:], scale=1.0,
                            )

                    Fs = []
                    for k in range(4):  # j chunk of 128
                        XT = xinp.tile([128, 256], i16, name="XT")
                        nc.sync.dma_start_transpose(XT[:], x_d[:, 128 * k:128 * (k + 1)])
                        XT1 = xinp.tile([128, 256], i16, name="XT1")
                        if k < 3:
                            nc.sync.dma_start_transpose(
                                XT1[:], x_d[:, 128 * k + 1:128 * (k + 1) + 1])
                        else:
                            # j=385..510 next byte = XT rows 1..127; j=511 row gets
                            # arbitrary defined data (its weight is always zero).
                            nc.sync.dma_start(XT1[0:127, :], XT[1:128, :])
                            nc.sync.dma_start(XT1[127:128, :], x_d[0:1, 0:256])
                        T8 = xinp.tile([128, 256], i16, name="T8")
                        nc.vector.tensor_scalar(T8[:], XT[:], 8, None, op0=Alu.logical_shift_left)
                        F = xinp.tile([128, 256], i16, name="F", bufs=8)
                        nc.vector.tensor_tensor(F[:], T8[:], XT1[:], op=Alu.bitwise_or)
                        Fs.append(F)

                    for k in range(4):
                        extract(k, 0)
                        extract(k, 1)
                        for q in range(8):
                            mm(q, 0, k, ps0[q])
                    for g in range(2):
                        ev = oevp.tile([128, 4, 512], f32, name="ev")
                        for qq in range(4):
                            q = 4 * g + qq
                            evict(q, 0, ps0[q], ev)
                        nc.gpsimd.dma_start(ot4[g, :, :, 0:512], ev[:])

                    for s in range(2, 8):
                        for k in range(4):
                            extract(k, s)
                        if s % 2 == 1:
                            n = s // 2
                            if n < 3:
                                for g in range(2):  # q quad
                                    ev = oevp.tile([128, 4, 512], f32, name="ev")
                                    for qq in range(4):
                                        q = 4 * g + qq
                                        ps = mpsum.tile([128, 512], f32, name="ps", tag="ps")
                                        for k in range(4):
                                            mm(q, n, k, ps)
                                        evict(q, n, ps, ev)
                                    nc.gpsimd.dma_start(
                                        ot4[g, :, :, 512 * n:512 * (n + 1)], ev[:]
                                    )
                            else:
                                # last chunk: small per-q DMAs for a short drain
                                for q in range(8):
                                    ps = mpsum.tile([128, 512], f32, name="ps", tag="ps")
                                    for k in range(4):
                                        mm(q, n, k, ps)
                                    ev1 = oevp.tile([128, 512], f32, name="ev1")
                                    nc.scalar.activation(
                                        ev1[:], ps[:],
                                        mybir.ActivationFunctionType.Identity,
                                        bias=bias_sb[:], scale=1.0,
                                    )
                                    eng = nc.sync if q % 2 == 0 else nc.scalar
                                    eng.dma_start(
                                        ot_flat[128 * q:128 * (q + 1), 512 * n:512 * (n + 1)],
                                        ev1[:],
                                    )

                if repeat == 1:
                    body()
                elif repeat < 0:  # unrolled (for cost-model experiments)
                    for _ in range(-repeat):
                        body()
                else:
                    with tc.For_i(0, repeat):
                        body()

    nc.finalize()
    return nc


def _prep_inputs(x, W):
    """Host-side prep: per-core int16 x views + replicated f16 weight tensors."""
    x_i16 = np.ascontiguousarray(x.astype(np.int16).reshape(B, 8, SUB))
    W = np.asarray(W, dtype=np.float32)
    Wsc = W * (2.0 / 255.0)

    # ws[j_local, q, k, m]: m = 64*t + o -> Wsc_pad_{2q+t}[128*k + j_local, o]
    wpad = np.zeros((16, 512, 64), np.float32)
    for p in range(16):
        wpad[p, p:p + 496, :] = Wsc
    ws = np.zeros((128, 4, 8, 128), np.float32)
    for q in range(8):
        for k in range(4):
            for t in range(2):
                ws[:, k, q, 64 * t:64 * t + 64] = wpad[2 * q + t, 128 * k:128 * (k + 1), :]
    ws = ws.astype(np.float16).reshape(128, 4096)

    bias = np.tile(-W.sum(0), 2).reshape(128, 1).astype(np.float32)  # -S, added

    in_maps = []
    for r in range(NCORES):
        xl = np.ascontiguousarray(
            x_i16[r * B_LOC:(r + 1) * B_LOC].reshape(B_LOC * 8, SUB)
        )
        in_maps.append({"x": xl, "ws": ws, "bias": bias})
    return in_maps


def _assemble(results):
    """Per-core OT [16,64,2048] -> (256,8,128,64) f32.

    OT column n = s*256 + bc, bc = 8*b_loc + c.
    """
    outs = []
    for r in range(NCORES):
        ot = np.asarray(results[r]["ot"], dtype=np.float32)
        o5 = ot.reshape(16, 64, 8, B_LOC, 8)          # [p, o, s, b_loc, c]
        outs.append(np.ascontiguousarray(
            o5.transpose(3, 4, 0, 2, 1)).reshape(B_LOC, 8, 128, 64))
    return np.concatenate(outs, axis=0)


def kernel(x, W):
    from concourse.bass_utils import run_bass_kernel_spmd

    if "nc" not in _CACHE:
        _CACHE["nc"] = _build_program(repeat=1)
    nc = _CACHE["nc"]
    in_maps = _prep_inputs(np.asarray(x), np.asarray(W))
    res = run_bass_kernel_spmd(nc, in_maps, core_ids=list(range(NCORES)))
    return _assemble(res.results)

